# revision 11
# baseline (speedup 1.0000x reference)
"""Trainium2 Bass kernel for nn_DecoderLSTM (gated-attention LSTM decoder).

Architecture (per core, SPMD over 8 NeuronCores):
  - Host: stable argsort by length (desc), caption gather, embedding gather,
    weight transposes/casts to fp16, vocab sharding of W_fc (2500 rows/core).
  - Device startup: encoder_out mean over P computed sharded (each core
    reduces its 256-column ENC slice), PE-transposed, AllGathered to the
    full [2048, 64] enc_mean in "A-layout" (ENC on partitions).
  - Device recurrence (replicated on all cores): 51 steps over the sorted
    ragged batch (active batch n_t shrinks with t).  All matmuls keep the
    output rows on partitions and the (shrinking) batch on the free dim:
        beta  = W_beta  @ h            [2048, n]
        awe   = sigmoid(beta + b_beta) * enc_mean      (fp16)
        gates = W_ihE @ awe + W_hh @ h + W_ihEmb @ e_t + b   [2048, n]
        cell  -> h_new [512, n] appended to the packed H store (fp16)
  - FC (vocab-sharded): logits for packed valid (t,b) columns only,
    lhsT = H chunks (fp16), rhs = W_fc^T slice, out [tb<=128, 500] PSUM,
    bias-added on eviction, DMA'd to the per-(t, b-run) slices of the
    pre-zeroed output.  Invalid (t,b) stay zero, matching the reference.

Numerics: fp16 matmul operands (PE runs fp16 at 1 cycle/row vs 4 for
fp32), fp32 PSUM accumulation and fp32 cell state / activations.
"""

import os
import sys

sys.path.insert(0, "/opt/trn_rl_repo")

import numpy as np

import concourse.bass as bass
import concourse.mybir as mybir
from concourse import tile
from concourse.bass_utils import run_bass_kernel_spmd

# ----------------------------------------------------------------------------
# Workaround: this walrus build accepts at most ONE sync wait per
# instruction, but Tile emits instructions (notably the kernel-tail drain)
# with several.  Rewrite the BIR JSON so excess waits move to same-engine
# NoOps inserted immediately before the over-full instruction.
# ----------------------------------------------------------------------------


def _split_waits_in_bir_json(bir: dict) -> dict:
    n_new = 0
    for fn in bir.get("functions", []):
        for blk in fn.get("blocks", []):
            out = []
            for ins in blk.get("instructions", []):
                si = ins.get("sync_info")
                waits = (si or {}).get("on_wait") or []
                if len(waits) > 1:
                    for w in waits[:-1]:
                        out.append(
                            {
                                "name": f"{ins['name']}-wsplit{n_new}",
                                "opcode": "NoOp",
                                "engine": ins["engine"],
                                "ins": [],
                                "outs": [],
                                "debug": None,
                                "sync_info": {"on_wait": [w], "on_update": []},
                            }
                        )
                        n_new += 1
                    si["on_wait"] = [waits[-1]]
                out.append(ins)
            blk["instructions"] = out
    return bir


def _install_patches():
    import orjson

    if not getattr(bass.Bass.to_json_bytes, "_wsplit_wrapped", False):
        orig = bass.Bass.to_json_bytes

        def to_json_bytes(self):
            bir = orjson.loads(orig(self))
            return orjson.dumps(_split_waits_in_bir_json(bir))

        to_json_bytes._wsplit_wrapped = True
        bass.Bass.to_json_bytes = to_json_bytes

    # trace=True support under axon: inject a minimal antenv.axon_hooks and
    # neuter the (bucket-less) artifact upload.  Only used when profiling.
    import types

    if "antenv.axon_hooks" not in sys.modules:
        mod = types.ModuleType("antenv.axon_hooks")
        mod._hook = None
        mod.set_axon_ntff_profile_hook = lambda h: setattr(mod, "_hook", h)
        mod.get_axon_ntff_profile_hook = lambda: mod._hook
        sys.modules["antenv.axon_hooks"] = mod
        try:
            import antenv

            antenv.axon_hooks = mod
            from trn_agent_boot.trn_boot import _ntff_profile_via_ctypes

            hook = _ntff_profile_via_ctypes("/opt/axon/libaxon_pjrt.so")
            if hook is not None:
                mod._hook = hook
        except Exception:
            pass

    import concourse.bass_utils as bu

    if not getattr(bu, "_upload_neutered", False):
        bu.upload_artifacts = lambda tmpdir: "local://" + str(tmpdir)
        bu._upload_neutered = True


_install_patches()

# ----------------------------------------------------------------------------

VOCAB, HID, EMB, ENC = 20000, 512, 300, 2048
B, P, L = 64, 196, 52
T = L - 1  # 51
N_CORES = 8
VC = VOCAB // N_CORES  # 2500 vocab rows per core
ECC = ENC // N_CORES  # 256 enc columns per core (mean reduction shard)
F32 = mybir.dt.float32
F16 = mybir.dt.float16

LAST_EXEC_NS = None


def _build_program(n_list, NT):
    """Build the SPMD bass program, specialized to the ragged step sizes."""
    steps = list(range(len(n_list)))  # t with n_t > 0 (prefix of 0..T-1)
    off = np.zeros(len(n_list) + 1, dtype=np.int64)
    off[1:] = np.cumsum(n_list)
    HW = NT + 64  # H store width: 64 zero cols + NT packed h_new cols

    nc = bass.Bass("TRN2", target_bir_lowering=False, debug=False,
                   num_devices=N_CORES)

    enc_p = nc.declare_dram_parameter("enc", [B, P, ECC], F32, isOutput=False)
    embT_p = nc.declare_dram_parameter("embT", [EMB, NT], F16, isOutput=False)
    wEmbT_p = nc.declare_dram_parameter("wEmbT", [EMB, 2048], F16, isOutput=False)
    wEncT_p = nc.declare_dram_parameter("wEncT", [ENC, 2048], F16, isOutput=False)
    wHhT_p = nc.declare_dram_parameter("wHhT", [HID, 2048], F16, isOutput=False)
    wBetaT_p = nc.declare_dram_parameter("wBetaT", [HID, 2048], F16, isOutput=False)
    wFcT_p = nc.declare_dram_parameter("wFcT", [HID, VC], F16, isOutput=False)
    bGatesF_p = nc.declare_dram_parameter("bGatesF", [1, 2048], F16, isOutput=False)
    bBetaF_p = nc.declare_dram_parameter("bBetaF", [1, 2048], F16, isOutput=False)
    bFc_p = nc.declare_dram_parameter("bFc", [128, VC], F16, isOutput=False)
    ident_p = nc.declare_dram_parameter("ident", [64, 64], F32, isOutput=False)
    pred_p = nc.declare_dram_parameter("pred", [B, T, VC], F32, isOutput=True)

    ag_in = nc.dram_tensor("ag_in", [2 * 128, B], F32)
    ag_out = nc.dram_tensor("ag_out", [ENC, B], F32, addr_space="Shared")

    debug = bool(int(os.environ.get("KERNEL_DEBUG", "0")))
    if debug:
        dbg_mean_p = nc.declare_dram_parameter("dbg_mean", [128, 16, 64], F32,
                                               isOutput=True)
        dbg_H_p = nc.declare_dram_parameter("dbg_H", [128, 4, min(256, HW)], F16,
                                            isOutput=True)
        dbg_awe_p = nc.declare_dram_parameter("dbg_awe", [128, 16, 64], F16,
                                              isOutput=True)

    with tile.TileContext(nc) as tc:
        with tc.tile_pool(name="betaP", bufs=1, space="PSUM") as betaPp, \
             tc.tile_pool(name="gatesP", bufs=1, space="PSUM") as gatesPp, \
             tc.tile_pool(name="fcP", bufs=2, space="PSUM") as fcPp, \
             tc.tile_pool(name="state", bufs=1) as state, \
             tc.tile_pool(name="work", bufs=1) as work:

            H = state.tile([128, 4, HW], F16)
            nc.vector.memset(H[:, :, 0:64], 0.0)
            cSt = state.tile([128, 4, 64], F32)
            nc.vector.memset(cSt[:], 0.0)
            ones_t = state.tile([1, 64], F16)
            nc.vector.memset(ones_t[:], 1.0)
            bGatesF = state.tile([1, 2048], F16)
            nc.sync.dma_start(out=bGatesF[:], in_=bGatesF_p[:])
            bBetaF = state.tile([1, 2048], F16)
            nc.sync.dma_start(out=bBetaF[:], in_=bBetaF_p[:])

            sigB = work.tile([128, 16, 64], F32)
            awe = work.tile([128, 16, 64], F16)
            gateA = work.tile([128, 16, 64], F32)
            tmp1 = work.tile([128, 4, 64], F32)
            tmp2 = work.tile([128, 4, 64], F32)
            tanhC = work.tile([128, 4, 64], F32)

            betaP = betaPp.tile([128, 16, 64], F32)
            gatesP = gatesPp.tile([128, 16, 64], F32)

            # ------ phase 0: enc mean (sharded) + transpose + AllGather ------
            with tc.tile_pool(name="encph", bufs=2) as encpool, \
                 tc.tile_pool(name="xp", bufs=1, space="PSUM") as xpool, \
                 tc.tile_pool(name="mean", bufs=1) as meanpool:
                meanAcc = meanpool.tile([64, ECC], F32)
                ident_t = meanpool.tile([64, 64], F32)
                nc.sync.dma_start(out=ident_t[:], in_=ident_p[:])
                for cc in range(ECC // 32):
                    et = encpool.tile([64, P, 32], F32, tag="enc")
                    nc.sync.dma_start(out=et[:],
                                      in_=enc_p[:, :, cc * 32:(cc + 1) * 32])
                    rv = et[:].rearrange("p a b -> p b a")  # [64, 32, P]
                    nc.vector.tensor_reduce(
                        meanAcc[:, cc * 32:(cc + 1) * 32], rv,
                        axis=mybir.AxisListType.X, op=mybir.AluOpType.add,
                    )
                nc.vector.tensor_scalar_mul(meanAcc[:], meanAcc[:], 1.0 / float(P))
                for h in range(ECC // 128):
                    pt = xpool.tile([128, 64], F32, tag="xp")
                    nc.tensor.transpose(
                        pt[:], meanAcc[:, h * 128:(h + 1) * 128], ident_t[:]
                    )
                    ev = encpool.tile([128, 64], F32, tag="ev")
                    nc.vector.tensor_copy(ev[:], pt[:])
                    nc.sync.dma_start(out=ag_in[h * 128:(h + 1) * 128, :],
                                      in_=ev[:])
            nc.gpsimd.collective_compute(
                "AllGather", mybir.AluOpType.bypass,
                ins=[ag_in[:]], outs=[ag_out[:]],
                replica_groups=[list(range(N_CORES))],
            )

            # ---------------- weights + recurrence + FC ----------------
            with tc.tile_pool(name="wts", bufs=1) as wts, \
                 tc.tile_pool(name="slab", bufs=2) as slabpool:
                wBeta = []
                for k in range(4):
                    t_ = wts.tile([128, 2048], F16, tag=f"wbeta{k}")
                    nc.sync.dma_start(out=t_[:],
                                      in_=wBetaT_p[k * 128:(k + 1) * 128, :])
                    wBeta.append(t_)
                wHh = []
                for k in range(4):
                    t_ = wts.tile([128, 2048], F16, tag=f"whh{k}")
                    nc.sync.dma_start(out=t_[:],
                                      in_=wHhT_p[k * 128:(k + 1) * 128, :])
                    wHh.append(t_)
                wEmb = []
                emb_ks = [128, 128, EMB - 256]
                for k in range(3):
                    t_ = wts.tile([128, 2048], F16, tag=f"wemb{k}")
                    nc.sync.dma_start(
                        out=t_[0:emb_ks[k], :],
                        in_=wEmbT_p[k * 128:k * 128 + emb_ks[k], :],
                    )
                    wEmb.append(t_)
                embT = []
                for k in range(3):
                    t_ = wts.tile([128, NT], F16, tag=f"embt{k}")
                    nc.sync.dma_start(
                        out=t_[0:emb_ks[k], :],
                        in_=embT_p[k * 128:k * 128 + emb_ks[k], :],
                    )
                    embT.append(t_)
                wEnc = []
                for k in range(16):
                    t_ = wts.tile([128, 2048], F16, tag=f"wenc{k}")
                    nc.sync.dma_start(out=t_[:],
                                      in_=wEncT_p[k * 128:(k + 1) * 128, :])
                    wEnc.append(t_)
                wFc = []
                for k in range(4):
                    t_ = wts.tile([128, VC], F16, tag=f"wfc{k}")
                    nc.sync.dma_start(out=t_[:],
                                      in_=wFcT_p[k * 128:(k + 1) * 128, :])
                    wFc.append(t_)
                bFc = wts.tile([128, VC], F16)
                nc.sync.dma_start(out=bFc[:], in_=bFc_p[:])
                encMean = wts.tile([128, 16, 64], F32)
                nc.sync.dma_start(
                    out=encMean[:],
                    in_=ag_out[:].rearrange("(m p) b -> p m b", p=128),
                )
                if debug:
                    nc.sync.dma_start(out=dbg_mean_p[:], in_=encMean[:])

                # packed column -> (t, b) map for FC output segments
                col2t = []
                for t in steps:
                    col2t.extend([t] * n_list[t])

                fc_done = 0

                def emit_fc_chunks(limit):
                    nonlocal fc_done
                    while fc_done < limit and (limit - fc_done >= 128
                                               or limit == NT):
                        c0 = fc_done
                        m = min(128, NT - c0)
                        slab = slabpool.tile([128, VC], F32, tag="slab")
                        for nn in range((VC + 499) // 500):
                            nw = min(500, VC - nn * 500)
                            ps = fcPp.tile([128, 500], F32, tag="fc")
                            for k in range(4):
                                nc.tensor.matmul(
                                    ps[0:m, 0:nw],
                                    lhsT=H[:, k, 64 + c0:64 + c0 + m],
                                    rhs=wFc[k][:, nn * 500:nn * 500 + nw],
                                    start=(k == 0), stop=(k == 3),
                                )
                            nc.vector.tensor_add(
                                slab[0:m, nn * 500:nn * 500 + nw],
                                ps[0:m, 0:nw],
                                bFc[0:m, nn * 500:nn * 500 + nw],
                            )
                        s = c0
                        while s < c0 + m:
                            t = col2t[s]
                            e = s
                            while e < c0 + m and col2t[e] == t:
                                e += 1
                            b0 = s - off[t]
                            nc.sync.dma_start(
                                out=pred_p[b0:b0 + (e - s), t, :],
                                in_=slab[s - c0:e - c0, :],
                            )
                            s = e
                        fc_done += m

                SIG = mybir.ActivationFunctionType.Sigmoid
                TANH = mybir.ActivationFunctionType.Tanh
                for t in steps:
                    n = int(n_list[t])
                    o_in = (64 + off[t - 1]) if t > 0 else 0
                    o_out = 64 + off[t]

                    def hs(k):
                        return H[:, k, o_in:o_in + n]

                    # beta = W_beta @ h + b_beta  -> [2048, n] in PSUM
                    for m in range(16):
                        nc.tensor.matmul(
                            betaP[:, m, 0:n],
                            lhsT=bBetaF[:, m * 128:(m + 1) * 128],
                            rhs=ones_t[:, 0:n],
                            start=True, stop=False,
                        )
                        for k in range(4):
                            nc.tensor.matmul(
                                betaP[:, m, 0:n],
                                lhsT=wBeta[k][:, m * 128:(m + 1) * 128],
                                rhs=hs(k),
                                start=False, stop=(k == 3),
                            )
                    # sigmoid straight from PSUM, in halves for pipelining
                    for h2 in range(2):
                        sl = slice(8 * h2, 8 * h2 + 8)
                        nc.scalar.activation(
                            sigB[:, sl, 0:n], betaP[:, sl, 0:n], SIG)
                        nc.vector.tensor_mul(
                            awe[:, sl, 0:n], sigB[:, sl, 0:n],
                            encMean[:, sl, 0:n])

                    # gates: bias + W_hh@h + W_ihEmb@e_t first, then W_ihE@awe
                    for m in range(16):
                        nc.tensor.matmul(
                            gatesP[:, m, 0:n],
                            lhsT=bGatesF[:, m * 128:(m + 1) * 128],
                            rhs=ones_t[:, 0:n],
                            start=True, stop=False,
                        )
                        for k in range(4):
                            nc.tensor.matmul(
                                gatesP[:, m, 0:n],
                                lhsT=wHh[k][:, m * 128:(m + 1) * 128],
                                rhs=hs(k),
                                start=False, stop=False,
                            )
                        for k in range(3):
                            nc.tensor.matmul(
                                gatesP[:, m, 0:n],
                                lhsT=wEmb[k][0:emb_ks[k], m * 128:(m + 1) * 128],
                                rhs=embT[k][0:emb_ks[k], off[t]:off[t] + n],
                                start=False, stop=False,
                            )
                        for k in range(16):
                            nc.tensor.matmul(
                                gatesP[:, m, 0:n],
                                lhsT=wEnc[k][:, m * 128:(m + 1) * 128],
                                rhs=awe[:, k, 0:n],
                                start=False, stop=(k == 15),
                            )

                    # cell: i = m0-3, f = m4-7, g = m8-11, o = m12-15
                    # activations straight from PSUM
                    nc.scalar.activation(gateA[:, 0:8, 0:n],
                                         gatesP[:, 0:8, 0:n], SIG)
                    nc.scalar.activation(gateA[:, 12:16, 0:n],
                                         gatesP[:, 12:16, 0:n], SIG)
                    nc.scalar.activation(gateA[:, 8:12, 0:n],
                                         gatesP[:, 8:12, 0:n], TANH)
                    nc.vector.tensor_mul(
                        tmp1[:, :, 0:n], gateA[:, 0:4, 0:n], gateA[:, 8:12, 0:n])
                    nc.vector.tensor_mul(
                        tmp2[:, :, 0:n], gateA[:, 4:8, 0:n], cSt[:, :, 0:n])
                    nc.vector.tensor_add(
                        cSt[:, :, 0:n], tmp1[:, :, 0:n], tmp2[:, :, 0:n])
                    nc.scalar.activation(tanhC[:, :, 0:n], cSt[:, :, 0:n], TANH)
                    nc.vector.tensor_mul(
                        H[:, :, o_out:o_out + n], gateA[:, 12:16, 0:n],
                        tanhC[:, :, 0:n])

                    if debug and t == 0:
                        nc.sync.dma_start(out=dbg_awe_p[:], in_=awe[:])

                    emit_fc_chunks(int(off[t] + n) if t != steps[-1] else NT)

                if debug:
                    nc.sync.dma_start(out=dbg_H_p[:],
                                      in_=H[:, :, 0:min(256, HW)])

    return nc


_CACHE = {}


def kernel(**inputs):
    global LAST_EXEC_NS
    enc_out = np.asarray(inputs["encoder_out"], dtype=np.float32)
    caps_in = np.asarray(inputs["encoded_captions"])
    cap_len = np.asarray(inputs["caption_lengths"])
    embedding = np.asarray(inputs["embedding"], dtype=np.float32)
    W_ih = np.asarray(inputs["W_ih"], dtype=np.float32)
    W_hh = np.asarray(inputs["W_hh"], dtype=np.float32)
    b_ih = np.asarray(inputs["b_ih"], dtype=np.float32)
    b_hh = np.asarray(inputs["b_hh"], dtype=np.float32)
    W_beta = np.asarray(inputs["W_beta"], dtype=np.float32)
    b_beta = np.asarray(inputs["b_beta"], dtype=np.float32)
    W_fc = np.asarray(inputs["W_fc"], dtype=np.float32)
    b_fc = np.asarray(inputs["b_fc"], dtype=np.float32)

    lengths = cap_len[:, 0]
    sort_ind = np.argsort(-lengths, kind="stable")
    lengths_s = lengths[sort_ind]
    caps = caps_in[sort_ind]
    dec = lengths_s - 1  # descending

    n_list = []
    for t in range(T):
        n = int((dec > t).sum())
        if n == 0:
            break
        n_list.append(n)
    NT = int(np.sum(n_list))
    off = np.zeros(len(n_list) + 1, dtype=np.int64)
    off[1:] = np.cumsum(n_list)

    key = (tuple(n_list), os.environ.get("KERNEL_DEBUG", "0"))
    if key not in _CACHE:
        _CACHE[key] = _build_program(n_list, NT)
    nc = _CACHE[key]

    # ---- host-side input prep ----
    emb_all = embedding[caps[:, :len(n_list)]]  # [B, Ts, EMB] f32
    embT = np.empty((EMB, NT), dtype=np.float16)
    for t, n in enumerate(n_list):
        embT[:, off[t]:off[t] + n] = emb_all[0:n, t, :].T
    wEmbT = np.ascontiguousarray(W_ih[:, :EMB].T).astype(np.float16)
    wEncT = np.ascontiguousarray(W_ih[:, EMB:].T).astype(np.float16)
    wHhT = np.ascontiguousarray(W_hh.T).astype(np.float16)
    wBetaT = np.ascontiguousarray(W_beta.T).astype(np.float16)
    bGatesF = (b_ih + b_hh)[None, :].astype(np.float16)
    bBetaF = b_beta[None, :].astype(np.float16)
    ident = np.eye(64, dtype=np.float32)
    enc_sorted = enc_out[sort_ind]  # [B, P, ENC]

    in_maps = []
    for j in range(N_CORES):
        v0 = j * VC
        in_maps.append({
            "enc": np.ascontiguousarray(enc_sorted[:, :, j * ECC:(j + 1) * ECC]),
            "embT": embT,
            "wEmbT": wEmbT,
            "wEncT": wEncT,
            "wHhT": wHhT,
            "wBetaT": wBetaT,
            "wFcT": np.ascontiguousarray(W_fc[v0:v0 + VC, :].T).astype(np.float16),
            "bGatesF": bGatesF,
            "bBetaF": bBetaF,
            "bFc": np.broadcast_to(
                b_fc[v0:v0 + VC].astype(np.float16), (128, VC)
            ).copy(),
            "ident": ident,
        })

    trace = bool(int(os.environ.get("KERNEL_TRACE", "0")))
    res = run_bass_kernel_spmd(nc, in_maps, list(range(N_CORES)), trace=trace)
    if trace:
        LAST_EXEC_NS = res.exec_time_ns

    predictions = np.zeros((B, T, VOCAB), dtype=np.float32)
    for j in range(N_CORES):
        predictions[:, :, j * VC:(j + 1) * VC] = res.results[j]["pred"]

    global LAST_DEBUG
    if os.environ.get("KERNEL_DEBUG", "0") == "1":
        LAST_DEBUG = {k: v for k, v in res.results[0].items() if k.startswith("dbg")}

    return (
        predictions,
        caps.astype(caps_in.dtype),
        dec.astype(cap_len.dtype),
        sort_ind.astype(np.int32),
    )


# revision 12
# speedup vs baseline: 1.3207x; 1.3207x over previous
"""Trainium2 Bass kernel for nn_DecoderLSTM (gated-attention LSTM decoder).

Architecture (per core, SPMD over 8 NeuronCores):
  - Host: stable argsort by length (desc), caption gather, embedding gather,
    weight transposes/casts to fp16, vocab sharding of W_fc (2500 rows/core).
  - Device startup: encoder_out mean over P computed sharded (each core
    reduces its 256-column ENC slice), PE-transposed, AllGathered to the
    full [2048, 64] enc_mean in "A-layout" (ENC on partitions).
  - Device recurrence (replicated on all cores): 51 steps over the sorted
    ragged batch (active batch n_t shrinks with t).  All matmuls keep the
    output rows on partitions and the (shrinking) batch on the free dim:
        beta  = W_beta  @ h            [2048, n]
        awe   = sigmoid(beta + b_beta) * enc_mean      (fp16)
        gates = W_ihE @ awe + W_hh @ h + W_ihEmb @ e_t + b   [2048, n]
        cell  -> h_new [512, n] appended to the packed H store (fp16)
  - FC (vocab-sharded): logits for packed valid (t,b) columns only,
    lhsT = H chunks (fp16), rhs = W_fc^T slice, out [tb<=128, 500] PSUM,
    bias-added on eviction, DMA'd to the per-(t, b-run) slices of the
    pre-zeroed output.  Invalid (t,b) stay zero, matching the reference.

Numerics: fp16 matmul operands (PE runs fp16 at 1 cycle/row vs 4 for
fp32), fp32 PSUM accumulation and fp32 cell state / activations.
"""

import os
import sys

sys.path.insert(0, "/opt/trn_rl_repo")

import numpy as np

import concourse.bass as bass
import concourse.mybir as mybir
from concourse import tile
from concourse.bass_utils import run_bass_kernel_spmd

# ----------------------------------------------------------------------------
# Workaround: this walrus build accepts at most ONE sync wait per
# instruction, but Tile emits instructions (notably the kernel-tail drain)
# with several.  Rewrite the BIR JSON so excess waits move to same-engine
# NoOps inserted immediately before the over-full instruction.
# ----------------------------------------------------------------------------


def _split_waits_in_bir_json(bir: dict) -> dict:
    n_new = 0
    for fn in bir.get("functions", []):
        for blk in fn.get("blocks", []):
            out = []
            for ins in blk.get("instructions", []):
                si = ins.get("sync_info")
                waits = (si or {}).get("on_wait") or []
                if len(waits) > 1:
                    for w in waits[:-1]:
                        out.append(
                            {
                                "name": f"{ins['name']}-wsplit{n_new}",
                                "opcode": "NoOp",
                                "engine": ins["engine"],
                                "ins": [],
                                "outs": [],
                                "debug": None,
                                "sync_info": {"on_wait": [w], "on_update": []},
                            }
                        )
                        n_new += 1
                    si["on_wait"] = [waits[-1]]
                out.append(ins)
            blk["instructions"] = out
    return bir


def _install_patches():
    import orjson

    if not getattr(bass.Bass.to_json_bytes, "_wsplit_wrapped", False):
        orig = bass.Bass.to_json_bytes

        def to_json_bytes(self):
            bir = orjson.loads(orig(self))
            return orjson.dumps(_split_waits_in_bir_json(bir))

        to_json_bytes._wsplit_wrapped = True
        bass.Bass.to_json_bytes = to_json_bytes

    # trace=True support under axon: inject a minimal antenv.axon_hooks and
    # neuter the (bucket-less) artifact upload.  Only used when profiling.
    import types

    if "antenv.axon_hooks" not in sys.modules:
        mod = types.ModuleType("antenv.axon_hooks")
        mod._hook = None
        mod.set_axon_ntff_profile_hook = lambda h: setattr(mod, "_hook", h)
        mod.get_axon_ntff_profile_hook = lambda: mod._hook
        sys.modules["antenv.axon_hooks"] = mod
        try:
            import antenv

            antenv.axon_hooks = mod
            from trn_agent_boot.trn_boot import _ntff_profile_via_ctypes

            hook = _ntff_profile_via_ctypes("/opt/axon/libaxon_pjrt.so")
            if hook is not None:
                mod._hook = hook
        except Exception:
            pass

    import concourse.bass_utils as bu

    if not getattr(bu, "_upload_neutered", False):
        bu.upload_artifacts = lambda tmpdir: "local://" + str(tmpdir)
        bu._upload_neutered = True


_install_patches()

# ----------------------------------------------------------------------------

VOCAB, HID, EMB, ENC = 20000, 512, 300, 2048
B, P, L = 64, 196, 52
T = L - 1  # 51
N_CORES = 8
VC = VOCAB // N_CORES  # 2500 vocab rows per core
ECC = ENC // N_CORES  # 256 enc columns per core (mean reduction shard)
F32 = mybir.dt.float32
F16 = mybir.dt.float16

LAST_EXEC_NS = None


def _build_program(n_list, NT, has_gbias, has_bbias):
    """Build the SPMD bass program, specialized to the ragged step sizes."""
    steps = list(range(len(n_list)))  # t with n_t > 0 (prefix of 0..T-1)
    off = np.zeros(len(n_list) + 1, dtype=np.int64)
    off[1:] = np.cumsum(n_list)
    HW = NT + 64  # H store width: 64 zero cols + NT packed h_new cols

    nc = bass.Bass("TRN2", target_bir_lowering=False, debug=False,
                   num_devices=N_CORES)

    enc_p = nc.declare_dram_parameter("enc", [B, P, ECC], F32, isOutput=False)
    embT_p = nc.declare_dram_parameter("embT", [EMB, NT], F16, isOutput=False)
    wEmbT_p = nc.declare_dram_parameter("wEmbT", [EMB, 2048], F16, isOutput=False)
    wEncT_p = nc.declare_dram_parameter("wEncT", [ENC, 2048], F16, isOutput=False)
    wHhT_p = nc.declare_dram_parameter("wHhT", [HID, 2048], F16, isOutput=False)
    wBetaT_p = nc.declare_dram_parameter("wBetaT", [HID, 2048], F16, isOutput=False)
    wFcT_p = nc.declare_dram_parameter("wFcT", [HID, VC], F16, isOutput=False)
    bGatesF_p = nc.declare_dram_parameter("bGatesF", [1, 2048], F16, isOutput=False)
    bBetaF_p = nc.declare_dram_parameter("bBetaF", [1, 2048], F16, isOutput=False)
    bFc_p = nc.declare_dram_parameter("bFc", [128, VC], F16, isOutput=False)
    ident_p = nc.declare_dram_parameter("ident", [64, 64], F32, isOutput=False)
    pred_p = nc.declare_dram_parameter("pred", [B, T, VC], F32, isOutput=True)

    ag_in = nc.dram_tensor("ag_in", [2 * 128, B], F32)
    ag_out = nc.dram_tensor("ag_out", [ENC, B], F32, addr_space="Shared")

    debug = bool(int(os.environ.get("KERNEL_DEBUG", "0")))
    if debug:
        dbg_mean_p = nc.declare_dram_parameter("dbg_mean", [128, 16, 64], F32,
                                               isOutput=True)
        dbg_H_p = nc.declare_dram_parameter("dbg_H", [128, 4, min(256, HW)], F16,
                                            isOutput=True)
        dbg_awe_p = nc.declare_dram_parameter("dbg_awe", [128, 16, 64], F16,
                                              isOutput=True)

    with tile.TileContext(nc) as tc:
        with tc.tile_pool(name="betaP", bufs=1, space="PSUM") as betaPp, \
             tc.tile_pool(name="gatesP", bufs=1, space="PSUM") as gatesPp, \
             tc.tile_pool(name="fcP", bufs=2, space="PSUM") as fcPp, \
             tc.tile_pool(name="state", bufs=1) as state, \
             tc.tile_pool(name="work", bufs=1) as work:

            H = state.tile([128, 4, HW], F16)
            nc.vector.memset(H[:, :, 0:64], 0.0)
            cSt = state.tile([128, 4, 64], F32)
            nc.vector.memset(cSt[:], 0.0)
            ones_t = state.tile([1, 64], F16)
            nc.vector.memset(ones_t[:], 1.0)
            bGatesF = state.tile([1, 2048], F16)
            nc.sync.dma_start(out=bGatesF[:], in_=bGatesF_p[:])
            bBetaF = state.tile([1, 2048], F16)
            nc.sync.dma_start(out=bBetaF[:], in_=bBetaF_p[:])

            sigB = work.tile([128, 16, 64], F32)
            awe = work.tile([128, 16, 64], F16)
            gateA = work.tile([128, 16, 64], F32)
            tmp1 = work.tile([128, 4, 64], F32)
            tmp2 = work.tile([128, 4, 64], F32)
            tanhC = work.tile([128, 4, 64], F32)

            betaP = betaPp.tile([128, 16, 64], F32)
            gatesP = gatesPp.tile([128, 16, 64], F32)

            # ------ phase 0: enc mean (sharded) + transpose + AllGather ------
            with tc.tile_pool(name="encph", bufs=2) as encpool, \
                 tc.tile_pool(name="xp", bufs=1, space="PSUM") as xpool, \
                 tc.tile_pool(name="mean", bufs=1) as meanpool:
                meanAcc = meanpool.tile([64, ECC], F32)
                ident_t = meanpool.tile([64, 64], F32)
                nc.sync.dma_start(out=ident_t[:], in_=ident_p[:])
                for cc in range(ECC // 32):
                    et = encpool.tile([64, P, 32], F32, tag="enc")
                    nc.sync.dma_start(out=et[:],
                                      in_=enc_p[:, :, cc * 32:(cc + 1) * 32])
                    rv = et[:].rearrange("p a b -> p b a")  # [64, 32, P]
                    nc.vector.tensor_reduce(
                        meanAcc[:, cc * 32:(cc + 1) * 32], rv,
                        axis=mybir.AxisListType.X, op=mybir.AluOpType.add,
                    )
                nc.vector.tensor_scalar_mul(meanAcc[:], meanAcc[:], 1.0 / float(P))
                for h in range(ECC // 128):
                    pt = xpool.tile([128, 64], F32, tag="xp")
                    nc.tensor.transpose(
                        pt[:], meanAcc[:, h * 128:(h + 1) * 128], ident_t[:]
                    )
                    ev = encpool.tile([128, 64], F32, tag="ev")
                    nc.vector.tensor_copy(ev[:], pt[:])
                    nc.sync.dma_start(out=ag_in[h * 128:(h + 1) * 128, :],
                                      in_=ev[:])
            nc.gpsimd.collective_compute(
                "AllGather", mybir.AluOpType.bypass,
                ins=[ag_in[:]], outs=[ag_out[:]],
                replica_groups=[list(range(N_CORES))],
            )

            # ---------------- weights + recurrence + FC ----------------
            with tc.tile_pool(name="wts", bufs=1) as wts, \
                 tc.tile_pool(name="slab", bufs=2) as slabpool:
                wBeta = []
                for k in range(4):
                    t_ = wts.tile([128, 2048], F16, tag=f"wbeta{k}")
                    nc.sync.dma_start(out=t_[:],
                                      in_=wBetaT_p[k * 128:(k + 1) * 128, :])
                    wBeta.append(t_)
                wHh = []
                for k in range(4):
                    t_ = wts.tile([128, 2048], F16, tag=f"whh{k}")
                    nc.sync.dma_start(out=t_[:],
                                      in_=wHhT_p[k * 128:(k + 1) * 128, :])
                    wHh.append(t_)
                wEmb = []
                emb_ks = [128, 128, EMB - 256]
                for k in range(3):
                    t_ = wts.tile([128, 2048], F16, tag=f"wemb{k}")
                    nc.sync.dma_start(
                        out=t_[0:emb_ks[k], :],
                        in_=wEmbT_p[k * 128:k * 128 + emb_ks[k], :],
                    )
                    wEmb.append(t_)
                embT = []
                for k in range(3):
                    t_ = wts.tile([128, NT], F16, tag=f"embt{k}")
                    nc.sync.dma_start(
                        out=t_[0:emb_ks[k], :],
                        in_=embT_p[k * 128:k * 128 + emb_ks[k], :],
                    )
                    embT.append(t_)
                wEnc = []
                for k in range(16):
                    t_ = wts.tile([128, 2048], F16, tag=f"wenc{k}")
                    nc.sync.dma_start(out=t_[:],
                                      in_=wEncT_p[k * 128:(k + 1) * 128, :])
                    wEnc.append(t_)
                wFc = []
                for k in range(4):
                    t_ = wts.tile([128, VC], F16, tag=f"wfc{k}")
                    nc.sync.dma_start(out=t_[:],
                                      in_=wFcT_p[k * 128:(k + 1) * 128, :])
                    wFc.append(t_)
                bFc = wts.tile([128, VC], F16)
                nc.sync.dma_start(out=bFc[:], in_=bFc_p[:])
                encMean = wts.tile([128, 16, 64], F32)
                nc.sync.dma_start(
                    out=encMean[:],
                    in_=ag_out[:].rearrange("(m p) b -> p m b", p=128),
                )
                if debug:
                    nc.sync.dma_start(out=dbg_mean_p[:], in_=encMean[:])

                # packed column -> (t, b) map for FC output segments
                col2t = []
                for t in steps:
                    col2t.extend([t] * n_list[t])

                fc_done = 0

                def emit_fc_chunks(limit):
                    nonlocal fc_done
                    while fc_done < limit and (limit - fc_done >= 128
                                               or limit == NT):
                        c0 = fc_done
                        m = min(128, NT - c0)
                        slab = slabpool.tile([128, VC], F32, tag="slab")
                        for nn in range((VC + 499) // 500):
                            nw = min(500, VC - nn * 500)
                            ps = fcPp.tile([128, 500], F32, tag="fc")
                            for k in range(4):
                                nc.tensor.matmul(
                                    ps[0:m, 0:nw],
                                    lhsT=H[:, k, 64 + c0:64 + c0 + m],
                                    rhs=wFc[k][:, nn * 500:nn * 500 + nw],
                                    start=(k == 0), stop=(k == 3),
                                )
                            nc.vector.tensor_add(
                                slab[0:m, nn * 500:nn * 500 + nw],
                                ps[0:m, 0:nw],
                                bFc[0:m, nn * 500:nn * 500 + nw],
                            )
                        s = c0
                        while s < c0 + m:
                            t = col2t[s]
                            e = s
                            while e < c0 + m and col2t[e] == t:
                                e += 1
                            b0 = s - off[t]
                            nc.sync.dma_start(
                                out=pred_p[b0:b0 + (e - s), t, :],
                                in_=slab[s - c0:e - c0, :],
                            )
                            s = e
                        fc_done += m

                SIG = mybir.ActivationFunctionType.Sigmoid
                TANH = mybir.ActivationFunctionType.Tanh
                for t in steps:
                    n = int(n_list[t])
                    o_in = (64 + off[t - 1]) if t > 0 else 0
                    o_out = 64 + off[t]

                    def hs(k):
                        return H[:, k, o_in:o_in + n]

                    # beta = W_beta @ h + b_beta  -> [2048, n] in PSUM
                    for m in range(16):
                        if has_bbias:
                            nc.tensor.matmul(
                                betaP[:, m, 0:n],
                                lhsT=bBetaF[:, m * 128:(m + 1) * 128],
                                rhs=ones_t[:, 0:n],
                                start=True, stop=False,
                            )
                        for k in range(4):
                            nc.tensor.matmul(
                                betaP[:, m, 0:n],
                                lhsT=wBeta[k][:, m * 128:(m + 1) * 128],
                                rhs=hs(k),
                                start=(k == 0 and not has_bbias),
                                stop=(k == 3),
                            )
                    # sigmoid straight from PSUM, in halves for pipelining
                    for h2 in range(2):
                        sl = slice(8 * h2, 8 * h2 + 8)
                        nc.scalar.activation(
                            sigB[:, sl, 0:n], betaP[:, sl, 0:n], SIG)
                        nc.vector.tensor_mul(
                            awe[:, sl, 0:n], sigB[:, sl, 0:n],
                            encMean[:, sl, 0:n])

                    # gates: bias + W_hh@h + W_ihEmb@e_t first, then W_ihE@awe
                    for m in range(16):
                        if has_gbias:
                            nc.tensor.matmul(
                                gatesP[:, m, 0:n],
                                lhsT=bGatesF[:, m * 128:(m + 1) * 128],
                                rhs=ones_t[:, 0:n],
                                start=True, stop=False,
                            )
                        for k in range(4):
                            nc.tensor.matmul(
                                gatesP[:, m, 0:n],
                                lhsT=wHh[k][:, m * 128:(m + 1) * 128],
                                rhs=hs(k),
                                start=(k == 0 and not has_gbias),
                                stop=False,
                            )
                        for k in range(3):
                            nc.tensor.matmul(
                                gatesP[:, m, 0:n],
                                lhsT=wEmb[k][0:emb_ks[k], m * 128:(m + 1) * 128],
                                rhs=embT[k][0:emb_ks[k], off[t]:off[t] + n],
                                start=False, stop=False,
                            )
                        for k in range(16):
                            nc.tensor.matmul(
                                gatesP[:, m, 0:n],
                                lhsT=wEnc[k][:, m * 128:(m + 1) * 128],
                                rhs=awe[:, k, 0:n],
                                start=False, stop=(k == 15),
                            )

                    # cell: i = m0-3, f = m4-7, g = m8-11, o = m12-15
                    # activations straight from PSUM
                    nc.scalar.activation(gateA[:, 0:8, 0:n],
                                         gatesP[:, 0:8, 0:n], SIG)
                    nc.scalar.activation(gateA[:, 12:16, 0:n],
                                         gatesP[:, 12:16, 0:n], SIG)
                    nc.scalar.activation(gateA[:, 8:12, 0:n],
                                         gatesP[:, 8:12, 0:n], TANH)
                    nc.vector.tensor_mul(
                        tmp1[:, :, 0:n], gateA[:, 0:4, 0:n], gateA[:, 8:12, 0:n])
                    nc.vector.tensor_mul(
                        tmp2[:, :, 0:n], gateA[:, 4:8, 0:n], cSt[:, :, 0:n])
                    nc.vector.tensor_add(
                        cSt[:, :, 0:n], tmp1[:, :, 0:n], tmp2[:, :, 0:n])
                    nc.scalar.activation(tanhC[:, :, 0:n], cSt[:, :, 0:n], TANH)
                    nc.vector.tensor_mul(
                        H[:, :, o_out:o_out + n], gateA[:, 12:16, 0:n],
                        tanhC[:, :, 0:n])

                    if debug and t == 0:
                        nc.sync.dma_start(out=dbg_awe_p[:], in_=awe[:])

                    emit_fc_chunks(int(off[t] + n) if t != steps[-1] else NT)

                if debug:
                    nc.sync.dma_start(out=dbg_H_p[:],
                                      in_=H[:, :, 0:min(256, HW)])

    return nc


_CACHE = {}


def kernel(**inputs):
    global LAST_EXEC_NS
    enc_out = np.asarray(inputs["encoder_out"], dtype=np.float32)
    caps_in = np.asarray(inputs["encoded_captions"])
    cap_len = np.asarray(inputs["caption_lengths"])
    embedding = np.asarray(inputs["embedding"], dtype=np.float32)
    W_ih = np.asarray(inputs["W_ih"], dtype=np.float32)
    W_hh = np.asarray(inputs["W_hh"], dtype=np.float32)
    b_ih = np.asarray(inputs["b_ih"], dtype=np.float32)
    b_hh = np.asarray(inputs["b_hh"], dtype=np.float32)
    W_beta = np.asarray(inputs["W_beta"], dtype=np.float32)
    b_beta = np.asarray(inputs["b_beta"], dtype=np.float32)
    W_fc = np.asarray(inputs["W_fc"], dtype=np.float32)
    b_fc = np.asarray(inputs["b_fc"], dtype=np.float32)

    lengths = cap_len[:, 0]
    sort_ind = np.argsort(-lengths, kind="stable")
    lengths_s = lengths[sort_ind]
    caps = caps_in[sort_ind]
    dec = lengths_s - 1  # descending

    n_list = []
    for t in range(T):
        n = int((dec > t).sum())
        if n == 0:
            break
        n_list.append(n)
    NT = int(np.sum(n_list))
    off = np.zeros(len(n_list) + 1, dtype=np.int64)
    off[1:] = np.cumsum(n_list)

    has_gbias = bool(np.any(b_ih) or np.any(b_hh))
    has_bbias = bool(np.any(b_beta))
    key = (tuple(n_list), has_gbias, has_bbias,
           os.environ.get("KERNEL_DEBUG", "0"))
    if key not in _CACHE:
        _CACHE[key] = _build_program(n_list, NT, has_gbias, has_bbias)
    nc = _CACHE[key]

    # ---- host-side input prep ----
    emb_all = embedding[caps[:, :len(n_list)]]  # [B, Ts, EMB] f32
    embT = np.empty((EMB, NT), dtype=np.float16)
    for t, n in enumerate(n_list):
        embT[:, off[t]:off[t] + n] = emb_all[0:n, t, :].T
    wEmbT = np.ascontiguousarray(W_ih[:, :EMB].T).astype(np.float16)
    wEncT = np.ascontiguousarray(W_ih[:, EMB:].T).astype(np.float16)
    wHhT = np.ascontiguousarray(W_hh.T).astype(np.float16)
    wBetaT = np.ascontiguousarray(W_beta.T).astype(np.float16)
    bGatesF = (b_ih + b_hh)[None, :].astype(np.float16)
    bBetaF = b_beta[None, :].astype(np.float16)
    ident = np.eye(64, dtype=np.float32)
    enc_sorted = enc_out[sort_ind]  # [B, P, ENC]

    in_maps = []
    for j in range(N_CORES):
        v0 = j * VC
        in_maps.append({
            "enc": np.ascontiguousarray(enc_sorted[:, :, j * ECC:(j + 1) * ECC]),
            "embT": embT,
            "wEmbT": wEmbT,
            "wEncT": wEncT,
            "wHhT": wHhT,
            "wBetaT": wBetaT,
            "wFcT": np.ascontiguousarray(W_fc[v0:v0 + VC, :].T).astype(np.float16),
            "bGatesF": bGatesF,
            "bBetaF": bBetaF,
            "bFc": np.broadcast_to(
                b_fc[v0:v0 + VC].astype(np.float16), (128, VC)
            ).copy(),
            "ident": ident,
        })

    trace = bool(int(os.environ.get("KERNEL_TRACE", "0")))
    res = run_bass_kernel_spmd(nc, in_maps, list(range(N_CORES)), trace=trace)
    if trace:
        LAST_EXEC_NS = res.exec_time_ns

    predictions = np.zeros((B, T, VOCAB), dtype=np.float32)
    for j in range(N_CORES):
        predictions[:, :, j * VC:(j + 1) * VC] = res.results[j]["pred"]

    global LAST_DEBUG
    if os.environ.get("KERNEL_DEBUG", "0") == "1":
        LAST_DEBUG = {k: v for k, v in res.results[0].items() if k.startswith("dbg")}

    return (
        predictions,
        caps.astype(caps_in.dtype),
        dec.astype(cap_len.dtype),
        sort_ind.astype(np.int32),
    )
